# revision 1
# baseline (speedup 1.0000x reference)
"""3-layer LSTM (B=256,T=512,I=256,H=512) + linear head on 8 NeuronCores.

Strategy: data-parallel over batch (32/core). Per layer, the input-side
matmul G = Wih @ x_t (+ biases) for a *chunk* of future time steps is
computed at full PE efficiency (N=512 streams) and interleaved with the
sequential h-recurrence of the current chunk; G never leaves SBUF.
Gate layout: gates.T packed [128 part, 512 cols] = 16 slots of 32 batch
cols in slot order i|f|o|g, all in ONE PSUM bank per step, preloaded
with G via one DVE copy, accumulated by 64 weight-stationary bf16
matmuls (K=128, M=128, N=32), then 2 ACT instrs (sigmoid over i|f|o,
tanh over g) evacuate to SBUF. c stays fp32-resident; h is written
bf16 directly into the layout the next matmul and the next layer's
batched input matmul consume.
"""

import numpy as np
import ml_dtypes
from contextlib import ExitStack

import concourse.bass as bass
import concourse.bacc as bacc
import concourse.tile as tile
from concourse import mybir
from concourse.bass_utils import run_bass_kernel_spmd

BF16 = mybir.dt.bfloat16
F32 = mybir.dt.float32
AF = mybir.ActivationFunctionType

B, T, I, H, O = 256, 512, 256, 512, 3
NCORES = 8
BL = B // NCORES          # 32 batch rows per core
SC = 16                   # time steps per chunk
CW = SC * BL              # 512 cols per chunk
NCH = T // SC             # 32 chunks
TOT = T * BL              # 16384 cols total
SLACK = 2 * CW            # prefetch overrun slack (cols)

# gate blocks in psum-slot order: i | f | o | g  (slot = blk*4 + j)
# block -> base row in the canonical (i,f,g,o) 2048 gate layout
GATE_BASE = [0, 512, 1536, 1024]   # i, f, o, g
KCS = [2, 4, 4]                    # K chunks per layer (256, 512, 512)


def _slot_row(slot):
    return GATE_BASE[slot // 4] + 128 * (slot % 4)


def _build():
    nc = bacc.Bacc("TRN2", target_bir_lowering=False, debug=False,
                   num_devices=NCORES)

    xt = nc.dram_tensor("x_t", (128, 2, TOT + SLACK), BF16, kind="ExternalInput")
    wih = [nc.dram_tensor(f"wih{l}", (128, KCS[l] * 2048), BF16,
                          kind="ExternalInput") for l in range(3)]
    whh = [nc.dram_tensor(f"whh{l}", (128, 4 * 2048), BF16,
                          kind="ExternalInput") for l in range(3)]
    bias_d = nc.dram_tensor("bias", (128, 48), F32, kind="ExternalInput")
    fcw_d = nc.dram_tensor("fcw", (128, 12), BF16, kind="ExternalInput")
    fcb_d = nc.dram_tensor("fcb", (3, 1), F32, kind="ExternalInput")
    out_d = nc.dram_tensor("out", (3, BL), F32, kind="ExternalOutput")

    with tile.TileContext(nc) as tc, ExitStack() as ctx:
        dram = ctx.enter_context(tc.tile_pool(name="dram", bufs=1, space="DRAM"))
        hdr = dram.tile([128, 4, TOT + SLACK], BF16)   # inter-layer H seq

        const = ctx.enter_context(tc.tile_pool(name="const", bufs=1))
        wih_sb = [const.tile([128, KCS[l] * 2048], BF16, tag=f"wih{l}",
                             name=f"wih_sb{l}") for l in range(3)]
        whh_sb = [const.tile([128, 4 * 2048], BF16, tag=f"whh{l}",
                             name=f"whh_sb{l}") for l in range(3)]
        bias_sb = const.tile([128, 48], F32, tag="bias")
        fcw_sb = const.tile([128, 12], BF16, tag="fcw")
        fcb_sb = const.tile([3, 1], F32, tag="fcb")
        for l in range(3):
            nc.sync.dma_start(wih_sb[l][:], wih[l].ap())
            nc.sync.dma_start(whh_sb[l][:], whh[l].ap())
        nc.sync.dma_start(bias_sb[:], bias_d.ap())
        nc.sync.dma_start(fcw_sb[:], fcw_d.ap())
        nc.sync.dma_start(fcb_sb[:], fcb_d.ap())

        big = ctx.enter_context(tc.tile_pool(name="big", bufs=1))
        g_buf = big.tile([128, 2 * 16 * CW], BF16, tag="gbuf")     # 4MB
        in_buf = big.tile([128, 4 * 4 * CW], BF16, tag="inbuf")    # 2MB
        h_stage = big.tile([128, 2 * 4 * CW], BF16, tag="hstage")  # 1MB
        c_t = big.tile([128, 128], F32, tag="cstate")

        g3 = g_buf[:].rearrange("p (s c) -> p s c", c=CW)    # [128, 32, CW]
        i3 = in_buf[:].rearrange("p (b c) -> p b c", c=CW)   # [128, 16, CW]
        h3 = h_stage[:].rearrange("p (x c) -> p x c", c=CW)  # [128, 8, CW]

        ew = ctx.enter_context(tc.tile_pool(name="ew", bufs=2))
        ps_rec = ctx.enter_context(tc.tile_pool(name="psr", bufs=2, space="PSUM"))
        ps_pa = ctx.enter_context(tc.tile_pool(name="psa", bufs=2, space="PSUM"))
        ps_fc = ctx.enter_context(tc.tile_pool(name="psf", bufs=1, space="PSUM"))

        def phase_a_slot(l, slot, in_base, g_base, in_ap):
            """G[slot] for one chunk: Kc matmuls (N=CW) + bias ACT."""
            kc = KCS[l]
            ps = ps_pa.tile([128, CW], F32, tag="pa")
            for k in range(kc):
                nc.tensor.matmul(
                    ps[:],
                    lhsT=wih_sb[l][:, k * 2048 + _slot_row(slot):
                                   k * 2048 + _slot_row(slot) + 128],
                    rhs=in_ap(in_base + k),
                    start=(k == 0), stop=(k == kc - 1),
                )
            nc.scalar.activation(
                g3[:, bass.ds(g_base + slot, 1), :].rearrange("p a c -> p (a c)"),
                ps[:], AF.Identity, bias=bias_sb[:, l * 16 + slot: l * 16 + slot + 1])

        def rec_step(l, s, g_base, h_rd, h_wr, pa_emit):
            """One recurrence time step; h_rd/h_wr are h3 block bases."""
            ps = ps_rec.tile([128, 512], F32, tag="rec")
            nc.vector.tensor_copy(
                ps[:].rearrange("p (a b) -> p a b", b=BL),
                g3[:, bass.ds(g_base, 16), s * BL: (s + 1) * BL])
            # h[t-1]: last slot of the other-parity buffer for s=0,
            # else slot s-1 of the current chunk's buffer
            hp_base = h_rd if s == 0 else h_wr
            hp_col = ((SC - 1) if s == 0 else (s - 1)) * BL
            for slot in range(16):
                for k in range(4):
                    nc.tensor.matmul(
                        ps[:, slot * BL:(slot + 1) * BL],
                        lhsT=whh_sb[l][:, k * 2048 + _slot_row(slot):
                                       k * 2048 + _slot_row(slot) + 128],
                        rhs=h3[:, bass.ds(hp_base + k, 1),
                               hp_col:hp_col + BL].rearrange("p a c -> p (a c)"),
                        start=False, stop=(k == 3), skip_group_check=True,
                    )
            gt = ew.tile([128, 512], F32, tag="gates")
            nc.scalar.activation(gt[:, 0:384], ps[:, 0:384], AF.Sigmoid)
            nc.scalar.activation(gt[:, 384:512], ps[:, 384:512], AF.Tanh)
            t1 = ew.tile([128, 128], F32, tag="t1")
            t2 = ew.tile([128, 128], F32, tag="t2")
            nc.vector.tensor_mul(t1[:], gt[:, 0:128], gt[:, 384:512])    # i*g
            nc.vector.tensor_mul(t2[:], gt[:, 128:256], c_t[:])          # f*c
            nc.vector.tensor_add(c_t[:], t1[:], t2[:])
            th = ew.tile([128, 128], F32, tag="th")
            nc.scalar.activation(th[:], c_t[:], AF.Tanh)
            nc.vector.tensor_mul(
                h3[:, bass.ds(h_wr, 4), s * BL:(s + 1) * BL],
                gt[:, 256:384].rearrange("p (a b) -> p a b", b=BL),
                th[:].rearrange("p (a b) -> p a b", b=BL))
            if pa_emit is not None:
                pa_emit(s)

        for l in range(3):
            in_dram = xt.ap() if l == 0 else hdr[:]
            kc = KCS[l]

            # prologue: In chunks 0,1 -> bufs 0,1 ; G chunk 0 -> parity 0
            nc.sync.dma_start(i3[:, 0:kc, :], in_dram[:, :, 0:CW])
            nc.sync.dma_start(i3[:, kc:2 * kc, :], in_dram[:, :, CW:2 * CW])
            for slot in range(16):
                phase_a_slot(l, slot, 0, 0,
                             lambda idx: i3[:, bass.ds(idx, 1), :]
                             .rearrange("p a c -> p (a c)"))
            nc.vector.memset(c_t[:], 0.0)
            nc.vector.memset(h3[:, bass.ds(4, 4), (SC - 1) * BL: SC * BL], 0.0)

            def body(iv, l=l, kc=kc, in_dram=in_dram):
                p2 = iv & 1
                q2 = (iv + 1) & 1
                ld_buf = ((iv + 2) & 3) * kc
                use_buf = ((iv + 1) & 3) * kc
                nc.sync.dma_start(
                    i3[:, bass.ds(ld_buf, kc), :],
                    in_dram[:, :, bass.ds((iv + 2) * CW, CW)])

                def pa_emit(s, l=l, use_buf=use_buf, q2=q2):
                    phase_a_slot(l, s, use_buf, q2 * 16,
                                 lambda idx: i3[:, bass.ds(idx, 1), :]
                                 .rearrange("p a c -> p (a c)"))

                for s in range(SC):
                    rec_step(l, s, p2 * 16, q2 * 4, p2 * 4, pa_emit)
                if l < 2:
                    nc.sync.dma_start(
                        hdr[:, :, bass.ds(iv * CW, CW)],
                        h3[:, bass.ds(p2 * 4, 4), :])

            with tc.For_i(0, NCH, 1) as iv:
                body(iv)

        # final linear head: out.T [3, BL] = fcW @ h_last (+ fcB)
        hb = ((NCH - 1) & 1) * 4
        ps = ps_fc.tile([3, BL], F32, tag="fc")
        for k in range(4):
            nc.tensor.matmul(
                ps[:], lhsT=fcw_sb[:, k * 3:(k + 1) * 3],
                rhs=h3[:, bass.ds(hb + k, 1), (SC - 1) * BL: SC * BL]
                .rearrange("p a c -> p (a c)"),
                start=(k == 0), stop=(k == 3))
        ob = ew.tile([3, BL], F32, tag="out")
        nc.scalar.activation(ob[:], ps[:], AF.Identity, bias=fcb_sb[:])
        nc.sync.dma_start(out_d.ap(), ob[:])

    nc.compile()
    return nc


def _prep(inputs):
    """Host-side layout prep. Returns per-core in_maps."""
    bf = ml_dtypes.bfloat16
    x = np.asarray(inputs["x"], np.float32)
    wihs = [np.asarray(inputs[f"Wih{l}"], np.float32) for l in range(3)]
    whhs = [np.asarray(inputs[f"Whh{l}"], np.float32) for l in range(3)]

    def wt_pack(w, kcs):  # [2048, K] -> [128, kcs*2048]
        return np.ascontiguousarray(
            w.T.reshape(kcs, 128, 2048).transpose(1, 0, 2)
            .reshape(128, kcs * 2048)).astype(bf)

    shared = {}
    for l in range(3):
        shared[f"wih{l}"] = wt_pack(wihs[l], KCS[l])
        shared[f"whh{l}"] = wt_pack(whhs[l], 4)
    bias = np.zeros((128, 48), np.float32)
    for l in range(3):
        bl_ = (np.asarray(inputs[f"bih{l}"], np.float32)
               + np.asarray(inputs[f"bhh{l}"], np.float32))
        for slot in range(16):
            r = _slot_row(slot)
            bias[:, l * 16 + slot] = bl_[r:r + 128]
    shared["bias"] = bias
    shared["fcw"] = np.ascontiguousarray(
        np.asarray(inputs["fcW"], np.float32).T.reshape(4, 128, 3)
        .transpose(1, 0, 2).reshape(128, 12)).astype(bf)
    shared["fcb"] = np.asarray(inputs["fcB"], np.float32).reshape(3, 1)

    in_maps = []
    for c in range(NCORES):
        xc = x[c * BL:(c + 1) * BL]                       # [32, 512, 256]
        xp = xc.transpose(2, 1, 0).reshape(2, 128, TOT)   # [2,128,16384]
        xp = np.ascontiguousarray(xp.transpose(1, 0, 2))  # [128,2,16384]
        xp = np.concatenate(
            [xp, np.zeros((128, 2, SLACK), np.float32)], axis=2).astype(bf)
        in_maps.append({"x_t": xp, **shared})
    return in_maps


_NC_CACHE = None


def kernel(**inputs):
    global _NC_CACHE
    if _NC_CACHE is None:
        _NC_CACHE = _build()
    nc = _NC_CACHE
    in_maps = _prep(inputs)
    res = run_bass_kernel_spmd(nc, in_maps, core_ids=list(range(NCORES)))
    out = np.empty((B, O), np.float32)
    for c in range(NCORES):
        out[c * BL:(c + 1) * BL] = res.results[c]["out"].T
    return out



# revision 3
# speedup vs baseline: 8.1112x; 8.1112x over previous
"""3-layer LSTM (B=256,T=512,I=256,H=512) + linear head on 8 NeuronCores.

Strategy: data-parallel over batch (32/core). Per step the gate matmul is
computed h-STATIONARY: the tiny h_t.T chunk ([128,32]) is the PE stationary
operand and the *weights* stream through the array as the moving operand,
split across the 4 PE column groups (tile_position=(0,32g)) so four N=512
weight streams run concurrently (~216ns per round of 2048 cols). This
removes the per-step LDWEIGHTS wall (8192 weight cols/step through the
slow load path) that bounds the weights-stationary form.

PSUM layout [32*slice+b, 128*q+jj] = gates.T: all 4 gates of a hidden unit
live on the same partition, so the sigmoid/tanh + cell update run with all
128 DVE/ACT lanes. h_t is transposed back to stationary form each step by
one [128,128] bf16 DMA-crossbar transpose (~1.2us, off the PE).

The 3 layers run as a wavefront (layer l computes step u-l in super-step u)
so the PE always has ~25 dense matmul rounds per super-step and the HAM
clock gate stays at 2.4GHz. h never leaves SBUF; biases enter via a K=1
ones-row matmul that also start=True-clears each psum accumulation group.
"""

import numpy as np
import ml_dtypes
from contextlib import ExitStack

import concourse.bass as bass
import concourse.bacc as bacc
import concourse.tile as tile
from concourse import mybir
from concourse.bass_utils import run_bass_kernel_spmd

BF16 = mybir.dt.bfloat16
F32 = mybir.dt.float32
AF = mybir.ActivationFunctionType

B, T, I, H, O = 256, 512, 256, 512, 3
NCORES = 8
BL = B // NCORES          # 32 batch rows per core
U = 5                     # super-steps per half-body (x chunk granularity)
NITER = 51                # steady loop iterations; covers u in [2, 512)
SLACK = 4 * U * 32        # x prefetch overrun slack (cols)

# per-layer input K chunks (x for l0: 256 = 2 chunks; h for l1/l2: 4 chunks)
KIN = [2, 4, 4]


def _build():
    nc = bacc.Bacc("TRN2", target_bir_lowering=False, debug=False,
                   num_devices=NCORES)

    xt = nc.dram_tensor("x_t", (128, 2, T * BL + SLACK), BF16,
                        kind="ExternalInput")
    # packed weight streams: [input chunks | whh chunks], each chunk 2048 cols
    wpk = [nc.dram_tensor(f"w{l}", (128, (KIN[l] + 4) * 2048), BF16,
                          kind="ExternalInput") for l in range(3)]
    bias_d = [nc.dram_tensor(f"b{l}", (1, 2048), BF16, kind="ExternalInput")
              for l in range(3)]
    fcw_d = nc.dram_tensor("fcw", (128, 12), BF16, kind="ExternalInput")
    fcb_d = nc.dram_tensor("fcb", (3, 1), F32, kind="ExternalInput")
    out_d = nc.dram_tensor("out", (3, BL), F32, kind="ExternalOutput")

    with tile.TileContext(nc) as tc, ExitStack() as ctx:
        const = ctx.enter_context(tc.tile_pool(name="const", bufs=1))
        w_sb = [const.tile([128, (KIN[l] + 4) * 2048], BF16, tag=f"w{l}",
                           name=f"w_sb{l}") for l in range(3)]
        bias_sb = [const.tile([1, 2048], BF16, tag=f"b{l}", name=f"bias_sb{l}")
                   for l in range(3)]
        ones_sb = const.tile([1, BL], BF16, tag="ones")
        fcw_sb = const.tile([128, 12], BF16, tag="fcw")
        fcb_sb = const.tile([3, 1], F32, tag="fcb")
        xboot = const.tile([128, 2, 2 * BL], BF16, tag="xboot")
        for l in range(3):
            nc.sync.dma_start(w_sb[l][:], wpk[l].ap())
            nc.sync.dma_start(bias_sb[l][:], bias_d[l].ap())
        nc.sync.dma_start(fcw_sb[:], fcw_d.ap())
        nc.sync.dma_start(fcb_sb[:], fcb_d.ap())
        nc.sync.dma_start(xboot[:], xt.ap()[:, :, 0:2 * BL])
        nc.vector.memset(ones_sb[:], 1.0)

        st = ctx.enter_context(tc.tile_pool(name="st", bufs=1))
        # persistent state
        c_st = [st.tile([128, 128], F32, tag=f"c{l}", name=f"c_st{l}")
                for l in range(3)]
        hT = [[st.tile([128, 128], BF16, tag=f"hT{l}_{p}", name=f"hT{l}_{p}")
               for p in range(2)] for l in range(3)]
        h_sb = [[st.tile([128, 128], BF16, tag=f"h{l}_{p}", name=f"h_sb{l}_{p}")
                 for p in range(2)] for l in range(3)]
        gs = [[st.tile([128, 512], F32, tag=f"gs{l}_{p}", name=f"gs{l}_{p}")
               for p in range(2)] for l in range(3)]
        xb = [st.tile([128, 2, U * BL], BF16, tag=f"xb{h}", name=f"xb{h}")
              for h in range(2)]

        for l in range(3):
            nc.vector.memset(c_st[l][:], 0.0)
            for p in range(2):
                nc.vector.memset(hT[l][p][:], 0.0)

        ew = ctx.enter_context(tc.tile_pool(name="ew", bufs=3))
        psp = ctx.enter_context(tc.tile_pool(name="psp", bufs=1, space="PSUM"))
        ps_g = [[psp.tile([128, 512], F32, tag=f"ps{l}_{p}", name=f"ps{l}_{p}")
                 for p in range(2)] for l in range(3)]
        ps_fc = psp.tile([3, BL], F32, tag="psfc")

        def mm_rounds(l, p, in_ap):
            """All matmul rounds for layer l, psum parity p.
            in_ap(kc) -> stationary [128,32] AP for input chunk kc."""
            ps = ps_g[l][p]
            kin = KIN[l]
            for g in range(4):
                nc.tensor.matmul(
                    ps[32 * g:32 * g + 32, :], lhsT=ones_sb[:],
                    rhs=bias_sb[l][:, 512 * g:512 * g + 512],
                    start=True, stop=False, tile_position=(0, 32 * g),
                    skip_group_check=True)
            for kc in range(kin + 4):
                lhsT = in_ap(kc) if kc < kin else \
                    hT[l][1 - p][:, 32 * (kc - kin):32 * (kc - kin) + 32]
                for g in range(4):
                    nc.tensor.matmul(
                        ps[32 * g:32 * g + 32, :], lhsT=lhsT,
                        rhs=w_sb[l][:, kc * 2048 + 512 * g:
                                    kc * 2048 + 512 * g + 512],
                        start=False, stop=(kc == kin + 3),
                        tile_position=(0, 32 * g), skip_group_check=True)

        def tail(l, p):
            """sigmoid/tanh + cell update + h transpose for layer l."""
            ps = ps_g[l][p]
            g_ = gs[l][p]
            nc.scalar.activation(g_[:, 128:256], ps[:, 128:256], AF.Sigmoid)
            nc.scalar.activation(g_[:, 0:128], ps[:, 0:128], AF.Sigmoid)
            nc.scalar.activation(g_[:, 256:384], ps[:, 256:384], AF.Tanh)
            nc.scalar.activation(g_[:, 384:512], ps[:, 384:512], AF.Sigmoid)
            t2 = ew.tile([128, 128], F32, tag="t2")
            nc.vector.tensor_mul(t2[:], g_[:, 128:256], c_st[l][:])
            t1 = ew.tile([128, 128], F32, tag="t1")
            nc.vector.tensor_mul(t1[:], g_[:, 0:128], g_[:, 256:384])
            nc.vector.tensor_add(c_st[l][:], t1[:], t2[:])
            th = ew.tile([128, 128], F32, tag="th")
            nc.scalar.activation(th[:], c_st[l][:], AF.Tanh)
            nc.vector.tensor_mul(h_sb[l][p][:], g_[:, 384:512], th[:])
            nc.sync.dma_start_transpose(hT[l][p][:], h_sb[l][p][:])

        def super_step(u_par, x_ap):
            """One super-step: layer l does its step; x_ap for layer 0."""
            p = u_par
            mm_rounds(0, p, x_ap)
            mm_rounds(1, p, lambda kc: hT[0][1 - p][:, 32 * kc:32 * kc + 32])
            mm_rounds(2, p, lambda kc: hT[1][1 - p][:, 32 * kc:32 * kc + 32])
            tail(0, p)
            tail(1, p)
            tail(2, p)

        # ---- peeled head: u=0 (l0 t=0), u=1 (l0 t=1, l1 t=0) ----
        def xboot_ap(t):
            return lambda kc: xboot[:, kc, t * BL:(t + 1) * BL]

        mm_rounds(0, 0, xboot_ap(0))
        tail(0, 0)
        mm_rounds(0, 1, xboot_ap(1))
        mm_rounds(1, 1, lambda kc: hT[0][0][:, 32 * kc:32 * kc + 32])
        tail(0, 1)
        tail(1, 1)

        # preload x: buf0 <- t [2,7), buf1 <- t [7,12)
        nc.sync.dma_start(xb[0][:], xt.ap()[:, :, 2 * BL:(2 + U) * BL])
        nc.sync.dma_start(xb[1][:], xt.ap()[:, :, (2 + U) * BL:(2 + 2 * U) * BL])

        # ---- steady loop: iteration iv covers u = 2+10iv .. 11+10iv ----
        def body(iv):
            for j in range(2 * U):
                half = j // U
                jj = j % U

                def x_ap(kc, half=half, jj=jj):
                    return xb[half][:, kc, jj * BL:(jj + 1) * BL]
                super_step(j & 1, x_ap)
                if j == U - 1:   # refill buf0 for next iteration
                    nc.sync.dma_start(
                        xb[0][:],
                        xt.ap()[:, :, bass.ds((iv * 2 * U + 2 + 2 * U) * BL,
                                              U * BL)])
                if j == 2 * U - 1:  # refill buf1 for next iteration
                    nc.sync.dma_start(
                        xb[1][:],
                        xt.ap()[:, :, bass.ds((iv * 2 * U + 2 + 3 * U) * BL,
                                              U * BL)])

        with tc.For_i(0, NITER, 1) as iv:
            body(iv)

        # ---- peeled tail: u=512 (l1 t=511, l2 t=510), u=513 (l2 t=511) ----
        # parity of u=512 is 0, u=513 is 1
        mm_rounds(1, 0, lambda kc: hT[0][1][:, 32 * kc:32 * kc + 32])
        mm_rounds(2, 0, lambda kc: hT[1][1][:, 32 * kc:32 * kc + 32])
        tail(1, 0)
        tail(2, 0)
        mm_rounds(2, 1, lambda kc: hT[1][0][:, 32 * kc:32 * kc + 32])
        tail(2, 1)

        # ---- head: out.T[3, BL] = fcW @ h2(511) + fcB ----
        for kc in range(4):
            nc.tensor.matmul(
                ps_fc[:], lhsT=fcw_sb[:, kc * 3:kc * 3 + 3],
                rhs=hT[2][1][:, 32 * kc:32 * kc + 32],
                start=(kc == 0), stop=(kc == 3))
        ob = ew.tile([3, BL], F32, tag="ob")
        nc.scalar.activation(ob[:], ps_fc[:], AF.Identity, bias=fcb_sb[:])
        nc.sync.dma_start(out_d.ap(), ob[:])

    nc.compile()
    return nc


def _prep(inputs):
    """Host-side layout prep. Returns per-core in_maps."""
    bf = ml_dtypes.bfloat16
    x = np.asarray(inputs["x"], np.float32)

    def stream_pack(w):
        """[2048, K] -> [128, (K/128)*2048] with col order (kc, g, q, jj):
        out[p, kc*2048 + g*512 + q*128 + jj] = w[512q+128g+jj, 128kc+p]."""
        K = w.shape[1]
        kcs = K // 128
        # w4[q, g, jj, kc, p]
        w4 = w.reshape(4, 4, 128, kcs, 128)
        # -> [p, kc, g, q, jj]
        return np.ascontiguousarray(
            w4.transpose(4, 3, 1, 0, 2).reshape(128, kcs * 2048)).astype(bf)

    shared = {}
    for l in range(3):
        wih = np.asarray(inputs[f"Wih{l}"], np.float32)
        whh = np.asarray(inputs[f"Whh{l}"], np.float32)
        shared[f"w{l}"] = np.concatenate(
            [stream_pack(wih), stream_pack(whh)], axis=1)
        bl_ = (np.asarray(inputs[f"bih{l}"], np.float32)
               + np.asarray(inputs[f"bhh{l}"], np.float32))
        # bias[0, g*512 + q*128 + jj] = bl_[512q+128g+jj]
        shared[f"b{l}"] = np.ascontiguousarray(
            bl_.reshape(4, 4, 128).transpose(1, 0, 2).reshape(1, 2048)
        ).astype(bf)
    shared["fcw"] = np.ascontiguousarray(
        np.asarray(inputs["fcW"], np.float32).T.reshape(4, 128, 3)
        .transpose(1, 0, 2).reshape(128, 12)).astype(bf)
    shared["fcb"] = np.asarray(inputs["fcB"], np.float32).reshape(3, 1)

    in_maps = []
    for c in range(NCORES):
        xc = x[c * BL:(c + 1) * BL]                       # [32, 512, 256]
        xp = xc.transpose(2, 1, 0).reshape(2, 128, T * BL)  # [2,128,16384]
        xp = np.ascontiguousarray(xp.transpose(1, 0, 2))    # [128,2,16384]
        xp = np.concatenate(
            [xp, np.zeros((128, 2, SLACK), np.float32)], axis=2).astype(bf)
        in_maps.append({"x_t": xp, **shared})
    return in_maps


_NC_CACHE = None


def kernel(**inputs):
    global _NC_CACHE
    if _NC_CACHE is None:
        _NC_CACHE = _build()
    nc = _NC_CACHE
    in_maps = _prep(inputs)
    res = run_bass_kernel_spmd(nc, in_maps, core_ids=list(range(NCORES)))
    out = np.empty((B, O), np.float32)
    for c in range(NCORES):
        out[c * BL:(c + 1) * BL] = res.results[c]["out"].T
    return out


# revision 5
# speedup vs baseline: 9.1874x; 1.1327x over previous
"""3-layer LSTM (B=256,T=512,I=256,H=512) + linear head on 8 NeuronCores.

Strategy: data-parallel over batch (32/core). Per step the gate matmul is
computed h-STATIONARY: the tiny h_t.T chunk ([128,32]) is the PE stationary
operand and the *weights* stream through the array as the moving operand,
split across the 4 PE column groups (tile_position=(0,32g)) so four N=512
weight streams run concurrently (~216ns per round of 2048 cols). This
removes the per-step LDWEIGHTS wall (8192 weight cols/step through the
slow load path) that bounds the weights-stationary form.

PSUM layout [32*slice+b, 128*q+jj] = gates.T: all 4 gates of a hidden unit
live on the same partition, so the sigmoid/tanh + cell update run with all
128 DVE/ACT lanes. h_t is transposed back to stationary form each step by
one [128,128] bf16 DMA-crossbar transpose (~1.2us, off the PE).

The 3 layers run as a wavefront (layer l computes step u-l in super-step u)
so the PE always has ~25 dense matmul rounds per super-step and the HAM
clock gate stays at 2.4GHz. h never leaves SBUF; biases enter via a K=1
ones-row matmul that also start=True-clears each psum accumulation group.
"""

import numpy as np
import ml_dtypes
from contextlib import ExitStack

import concourse.bass as bass
import concourse.bacc as bacc
import concourse.tile as tile
from concourse import mybir
from concourse.bass_utils import run_bass_kernel_spmd

BF16 = mybir.dt.bfloat16
F32 = mybir.dt.float32
AF = mybir.ActivationFunctionType

B, T, I, H, O = 256, 512, 256, 512, 3
NCORES = 8
BL = B // NCORES          # 32 batch rows per core
U = 15                    # super-steps per half-body (x chunk granularity)
NITER = 17                # steady loop iterations; covers u in [2, 512)
SLACK = 4 * U * 32        # x prefetch overrun slack (cols)

# per-layer input K chunks (x for l0: 256 = 2 chunks; h for l1/l2: 4 chunks)
KIN = [2, 4, 4]


def _build():
    nc = bacc.Bacc("TRN2", target_bir_lowering=False, debug=False,
                   num_devices=NCORES)

    xt = nc.dram_tensor("x_t", (128, 2, T * BL + SLACK), BF16,
                        kind="ExternalInput")
    # packed weight streams: [input chunks | whh chunks], each chunk 2048 cols
    wpk = [nc.dram_tensor(f"w{l}", (128, (KIN[l] + 4) * 2048), BF16,
                          kind="ExternalInput") for l in range(3)]
    bias_d = [nc.dram_tensor(f"b{l}", (1, 2048), BF16, kind="ExternalInput")
              for l in range(3)]
    fcw_d = nc.dram_tensor("fcw", (128, 12), BF16, kind="ExternalInput")
    fcb_d = nc.dram_tensor("fcb", (3, 1), F32, kind="ExternalInput")
    out_d = nc.dram_tensor("out", (3, BL), F32, kind="ExternalOutput")

    with tile.TileContext(nc) as tc, ExitStack() as ctx:
        const = ctx.enter_context(tc.tile_pool(name="const", bufs=1))
        w_sb = [const.tile([128, (KIN[l] + 4) * 2048], BF16, tag=f"w{l}",
                           name=f"w_sb{l}") for l in range(3)]
        bias_sb = [const.tile([1, 2048], BF16, tag=f"b{l}", name=f"bias_sb{l}")
                   for l in range(3)]
        ones_sb = const.tile([1, BL], BF16, tag="ones")
        fcw_sb = const.tile([128, 12], BF16, tag="fcw")
        fcb_sb = const.tile([3, 1], F32, tag="fcb")
        xboot = const.tile([128, 2, 2 * BL], BF16, tag="xboot")
        for l in range(3):
            nc.sync.dma_start(w_sb[l][:], wpk[l].ap())
            nc.sync.dma_start(bias_sb[l][:], bias_d[l].ap())
        nc.sync.dma_start(fcw_sb[:], fcw_d.ap())
        nc.sync.dma_start(fcb_sb[:], fcb_d.ap())
        nc.sync.dma_start(xboot[:], xt.ap()[:, :, 0:2 * BL])
        nc.vector.memset(ones_sb[:], 1.0)

        st = ctx.enter_context(tc.tile_pool(name="st", bufs=1))
        # persistent state
        c_st = [st.tile([128, 128], F32, tag=f"c{l}", name=f"c_st{l}")
                for l in range(3)]
        hT = [[st.tile([128, 128], BF16, tag=f"hT{l}_{p}", name=f"hT{l}_{p}")
               for p in range(2)] for l in range(3)]
        h_sb = [[st.tile([128, 128], BF16, tag=f"h{l}_{p}", name=f"h_sb{l}_{p}")
                 for p in range(2)] for l in range(3)]
        gs = [[st.tile([128, 512], F32, tag=f"gs{l}_{p}", name=f"gs{l}_{p}")
               for p in range(2)] for l in range(3)]
        xb = [st.tile([128, 2, U * BL], BF16, tag=f"xb{h}", name=f"xb{h}")
              for h in range(2)]

        for l in range(3):
            nc.vector.memset(c_st[l][:], 0.0)
            for p in range(2):
                nc.vector.memset(hT[l][p][:], 0.0)

        ew = ctx.enter_context(tc.tile_pool(name="ew", bufs=3))
        psp = ctx.enter_context(tc.tile_pool(name="psp", bufs=1, space="PSUM"))
        ps_g = [[psp.tile([128, 512], F32, tag=f"ps{l}_{p}", name=f"ps{l}_{p}")
                 for p in range(2)] for l in range(3)]
        ps_fc = psp.tile([3, BL], F32, tag="psfc")

        def mm_rounds(l, p, in_ap):
            """All matmul rounds for layer l, psum parity p.
            in_ap(kc) -> stationary [128,32] AP for input chunk kc."""
            ps = ps_g[l][p]
            kin = KIN[l]
            for g in range(4):
                nc.tensor.matmul(
                    ps[32 * g:32 * g + 32, :], lhsT=ones_sb[:],
                    rhs=bias_sb[l][:, 512 * g:512 * g + 512],
                    start=True, stop=False, tile_position=(0, 32 * g),
                    skip_group_check=True)
            for kc in range(kin + 4):
                lhsT = in_ap(kc) if kc < kin else \
                    hT[l][1 - p][:, 32 * (kc - kin):32 * (kc - kin) + 32]
                for g in range(4):
                    nc.tensor.matmul(
                        ps[32 * g:32 * g + 32, :], lhsT=lhsT,
                        rhs=w_sb[l][:, kc * 2048 + 512 * g:
                                    kc * 2048 + 512 * g + 512],
                        start=False, stop=(kc == kin + 3),
                        tile_position=(0, 32 * g), skip_group_check=True)

        def tail(l, p):
            """sigmoid/tanh + cell update + h transpose for layer l."""
            ps = ps_g[l][p]
            g_ = gs[l][p]
            nc.scalar.activation(g_[:, 0:256], ps[:, 0:256], AF.Sigmoid)
            nc.scalar.activation(g_[:, 256:384], ps[:, 256:384], AF.Tanh)
            nc.scalar.activation(g_[:, 384:512], ps[:, 384:512], AF.Sigmoid)
            t2 = ew.tile([128, 128], F32, tag="t2")
            nc.vector.tensor_mul(t2[:], g_[:, 128:256], c_st[l][:])
            t1 = ew.tile([128, 128], F32, tag="t1")
            nc.vector.tensor_mul(t1[:], g_[:, 0:128], g_[:, 256:384])
            nc.vector.tensor_add(c_st[l][:], t1[:], t2[:])
            th = ew.tile([128, 128], F32, tag="th")
            nc.scalar.activation(th[:], c_st[l][:], AF.Tanh)
            nc.vector.tensor_mul(h_sb[l][p][:], g_[:, 384:512], th[:])
            nc.sync.dma_start_transpose(hT[l][p][:], h_sb[l][p][:])

        def super_step(u_par, x_ap):
            """One super-step: layer l does its step; x_ap for layer 0."""
            p = u_par
            mm_rounds(0, p, x_ap)
            mm_rounds(1, p, lambda kc: hT[0][1 - p][:, 32 * kc:32 * kc + 32])
            mm_rounds(2, p, lambda kc: hT[1][1 - p][:, 32 * kc:32 * kc + 32])
            tail(0, p)
            tail(1, p)
            tail(2, p)

        # ---- peeled head: u=0 (l0 t=0), u=1 (l0 t=1, l1 t=0) ----
        def xboot_ap(t):
            return lambda kc: xboot[:, kc, t * BL:(t + 1) * BL]

        mm_rounds(0, 0, xboot_ap(0))
        tail(0, 0)
        mm_rounds(0, 1, xboot_ap(1))
        mm_rounds(1, 1, lambda kc: hT[0][0][:, 32 * kc:32 * kc + 32])
        tail(0, 1)
        tail(1, 1)

        # preload x: buf0 <- t [2,7), buf1 <- t [7,12)
        nc.sync.dma_start(xb[0][:], xt.ap()[:, :, 2 * BL:(2 + U) * BL])
        nc.sync.dma_start(xb[1][:], xt.ap()[:, :, (2 + U) * BL:(2 + 2 * U) * BL])

        # ---- steady loop: iteration iv covers u = 2+10iv .. 11+10iv ----
        def body(iv):
            for j in range(2 * U):
                half = j // U
                jj = j % U

                def x_ap(kc, half=half, jj=jj):
                    return xb[half][:, kc, jj * BL:(jj + 1) * BL]
                super_step(j & 1, x_ap)
                if j == U - 1:   # refill buf0 for next iteration
                    nc.scalar.dma_start(
                        xb[0][:],
                        xt.ap()[:, :, bass.ds((iv * 2 * U + 2 + 2 * U) * BL,
                                              U * BL)])
                if j == 2 * U - 1:  # refill buf1 for next iteration
                    nc.scalar.dma_start(
                        xb[1][:],
                        xt.ap()[:, :, bass.ds((iv * 2 * U + 2 + 3 * U) * BL,
                                              U * BL)])

        with tc.For_i(0, NITER, 1,
                      hint_engines=(mybir.EngineType.PE,
                                    mybir.EngineType.Activation,
                                    mybir.EngineType.DVE)) as iv:
            body(iv)

        # ---- peeled tail: u=512 (l1 t=511, l2 t=510), u=513 (l2 t=511) ----
        # parity of u=512 is 0, u=513 is 1
        mm_rounds(1, 0, lambda kc: hT[0][1][:, 32 * kc:32 * kc + 32])
        mm_rounds(2, 0, lambda kc: hT[1][1][:, 32 * kc:32 * kc + 32])
        tail(1, 0)
        tail(2, 0)
        mm_rounds(2, 1, lambda kc: hT[1][0][:, 32 * kc:32 * kc + 32])
        tail(2, 1)

        # ---- head: out.T[3, BL] = fcW @ h2(511) + fcB ----
        for kc in range(4):
            nc.tensor.matmul(
                ps_fc[:], lhsT=fcw_sb[:, kc * 3:kc * 3 + 3],
                rhs=hT[2][1][:, 32 * kc:32 * kc + 32],
                start=(kc == 0), stop=(kc == 3))
        ob = ew.tile([3, BL], F32, tag="ob")
        nc.scalar.activation(ob[:], ps_fc[:], AF.Identity, bias=fcb_sb[:])
        nc.sync.dma_start(out_d.ap(), ob[:])

    nc.compile()
    return nc


def _prep(inputs):
    """Host-side layout prep. Returns per-core in_maps."""
    bf = ml_dtypes.bfloat16
    x = np.asarray(inputs["x"], np.float32)

    def stream_pack(w):
        """[2048, K] -> [128, (K/128)*2048] with col order (kc, g, q, jj):
        out[p, kc*2048 + g*512 + q*128 + jj] = w[512q+128g+jj, 128kc+p]."""
        K = w.shape[1]
        kcs = K // 128
        # w4[q, g, jj, kc, p]
        w4 = w.reshape(4, 4, 128, kcs, 128)
        # -> [p, kc, g, q, jj]
        return np.ascontiguousarray(
            w4.transpose(4, 3, 1, 0, 2).reshape(128, kcs * 2048)).astype(bf)

    shared = {}
    for l in range(3):
        wih = np.asarray(inputs[f"Wih{l}"], np.float32)
        whh = np.asarray(inputs[f"Whh{l}"], np.float32)
        shared[f"w{l}"] = np.concatenate(
            [stream_pack(wih), stream_pack(whh)], axis=1)
        bl_ = (np.asarray(inputs[f"bih{l}"], np.float32)
               + np.asarray(inputs[f"bhh{l}"], np.float32))
        # bias[0, g*512 + q*128 + jj] = bl_[512q+128g+jj]
        shared[f"b{l}"] = np.ascontiguousarray(
            bl_.reshape(4, 4, 128).transpose(1, 0, 2).reshape(1, 2048)
        ).astype(bf)
    shared["fcw"] = np.ascontiguousarray(
        np.asarray(inputs["fcW"], np.float32).T.reshape(4, 128, 3)
        .transpose(1, 0, 2).reshape(128, 12)).astype(bf)
    shared["fcb"] = np.asarray(inputs["fcB"], np.float32).reshape(3, 1)

    in_maps = []
    for c in range(NCORES):
        xc = x[c * BL:(c + 1) * BL]                       # [32, 512, 256]
        xp = xc.transpose(2, 1, 0).reshape(2, 128, T * BL)  # [2,128,16384]
        xp = np.ascontiguousarray(xp.transpose(1, 0, 2))    # [128,2,16384]
        xp = np.concatenate(
            [xp, np.zeros((128, 2, SLACK), np.float32)], axis=2).astype(bf)
        in_maps.append({"x_t": xp, **shared})
    return in_maps


_NC_CACHE = None


def kernel(**inputs):
    global _NC_CACHE
    if _NC_CACHE is None:
        _NC_CACHE = _build()
    nc = _NC_CACHE
    in_maps = _prep(inputs)
    res = run_bass_kernel_spmd(nc, in_maps, core_ids=list(range(NCORES)))
    out = np.empty((B, O), np.float32)
    for c in range(NCORES):
        out[c * BL:(c + 1) * BL] = res.results[c]["out"].T
    return out
